# revision 1
# baseline (speedup 1.0000x reference)
"""Causal self-attention (B=4, T=2048, C=1024, H=16) on 8 trn2 NeuronCores.

Sharding: core c -> (batch b = c//2, head-group g = c%2 of 8 heads).
Each core computes qkv projection, causal attention and the proj partial-sum
for its 8 heads on its batch; the host sums the two head-group partials per
batch (row-parallel linear unshard).

Per-core kernel layout (all on-device matmuls bf16, f32 accumulation):
  xT [C, T] (pre-transposed on host) so QKV contraction runs with c on the
  partition axis with zero on-device transposes.
  QT/KT [2*64, T] per head pair -> scores S_T[t_k, t_q] via two k=64 matmuls
  packed into PE row-groups 0-63/64-127 (tile_position auto-derivation).
  Softmax without max-subtraction (logits ~ N(0,1), fp32-safe); denominator
  via an appended ones-column in the AV lhsT (m=65, row 64 = sum of exp).
  exp on ScalarE in [128, 2048] mega-tiles (bf16 PSUM) to amortize overhead.
  Normalization: reciprocal_approx_fast of denoms + GpSimd partition
  broadcast + one in-place multiply per pair; proj with k=128 chunks.
"""

from contextlib import ExitStack

import ml_dtypes
import numpy as np
import orjson

import concourse.bass as bass
import concourse.mybir as mybir
import concourse.tile as tile
from concourse.bass_utils import run_bass_kernel_spmd

BF16 = mybir.dt.bfloat16
F32 = mybir.dt.float32
F32R = mybir.dt.float32r
AF = mybir.ActivationFunctionType

T, C, H, DH = 2048, 1024, 16, 64
NCORES = 8
NPAIR = 4            # head pairs per core (8 heads)
CCH = C // 128       # contraction chunks for qkv
TQ = T // 512        # query chunks
NT = T // 128        # token tiles
VROW = 65            # 64 v-cols + ones column

# --- walrus in this env accepts only ONE sync-wait per instruction: split
# extras onto preceding same-engine NoOps at the BIR-JSON level.
if not getattr(bass.Bass, "_ant_wait_split", False):
    _orig_to_json_bytes = bass.Bass.to_json_bytes

    def _to_json_split_waits(self):
        m = orjson.loads(_orig_to_json_bytes(self))
        for f in m.get("functions", []):
            for bb in f.get("blocks") or []:
                insts = bb.get("instructions") or []
                out, changed = [], False
                for inst in insts:
                    si = inst.get("sync_info")
                    waits = (si or {}).get("on_wait") or []
                    if len(waits) > 1:
                        for j, w in enumerate(waits[:-1]):
                            out.append({
                                "debug": inst.get("debug", 0),
                                "engine": inst["engine"],
                                "ins": [], "outs": [],
                                "name": f"{inst['name']}-sw{j}",
                                "opcode": "NoOp",
                                "sync_info": {"on_wait": [w], "on_update": []},
                            })
                        si["on_wait"] = waits[-1:]
                        changed = True
                    out.append(inst)
                if changed:
                    bb["instructions"] = out
        return orjson.dumps(m)

    bass.Bass.to_json_bytes = _to_json_split_waits
    bass.Bass._ant_wait_split = True


def build_program() -> bass.Bass:
    nc = bass.Bass()
    xT = nc.dram_tensor("xT", [C, T], BF16, kind="ExternalInput")
    wqkvT = nc.dram_tensor("wqkvT", [C, 1536], BF16, kind="ExternalInput")
    wpT = nc.dram_tensor("wpT", [512, C], BF16, kind="ExternalInput")
    dmask = nc.dram_tensor("dmask", [128, 2048], BF16, kind="ExternalInput")
    seld = nc.dram_tensor("sel", [2, 128], F32R, kind="ExternalInput")
    out = nc.dram_tensor("out", [T, C], F32, kind="ExternalOutput")

    with ExitStack() as ctx:
        tc = ctx.enter_context(tile.TileContext(nc))
        const = ctx.enter_context(tc.tile_pool(name="const", bufs=1))
        pss = ctx.enter_context(tc.tile_pool(name="pss", bufs=2, space="PSUM"))
        psy = ctx.enter_context(tc.tile_pool(name="psy", bufs=2, space="PSUM"))
        ppool = ctx.enter_context(tc.tile_pool(name="ppool", bufs=3))
        spool = ctx.enter_context(tc.tile_pool(name="spool", bufs=2))
        rbpool = ctx.enter_context(tc.tile_pool(name="rbpool", bufs=2))
        opool = ctx.enter_context(tc.tile_pool(name="opool", bufs=2))
        dram = ctx.enter_context(tc.tile_pool(name="dram", bufs=1, space="DRAM"))
        dstage = dram.tile([16, 1024], F32, tag="dstage")
        rstage = dram.tile([16, 1024], F32, tag="rstage")

        xT_sb = const.tile([128, CCH, T], BF16, tag="xT")
        wq_sb = const.tile([128, CCH, 1536], BF16, tag="wq")
        wp_sb = const.tile([128, 4, C], BF16, tag="wp")
        dm_sb = const.tile([128, 2048], BF16, tag="dm")
        QT_sb = const.tile([128, NPAIR, T], BF16, tag="QT")
        KT_sb = const.tile([128, NPAIR, T], BF16, tag="KT")
        V_sb = const.tile([128, NT, 8 * VROW], BF16, tag="V")
        Yu_sb = const.tile([128, NPAIR, T], BF16, tag="Yu")

        for c in range(CCH):
            # split halves across DMA queues for a faster input ramp
            nc.sync.dma_start(xT_sb[:, c, 0:1024], xT[c * 128:(c + 1) * 128, 0:1024])
            nc.sync.dma_start(xT_sb[:, c, 1024:2048], xT[c * 128:(c + 1) * 128, 1024:2048])
            nc.sync.dma_start(wq_sb[:, c, 0:768], wqkvT[c * 128:(c + 1) * 128, 0:768])
            nc.sync.dma_start(wq_sb[:, c, 768:1536], wqkvT[c * 128:(c + 1) * 128, 768:1536])
        for c in range(4):
            nc.sync.dma_start(wp_sb[:, c, :], wpT[c * 128:(c + 1) * 128, :])
        nc.sync.dma_start(dm_sb[:], dmask[:])

        vr = V_sb[:].rearrange("p n (h e) -> p n h e", e=VROW)
        nc.gpsimd.memset(vr[:, :, :, 64:65], 1.0)
        # selector for the k=2 reciprocal-broadcast matmul:
        # out[m,:] = sel[0,m]*rt[0,:] + sel[1,m]*rt[1,:] -> A rows 0-63, B rows 64-127
        sel_sb = const.tile([128, 128], F32R, tag="sel")
        nc.sync.dma_start(sel_sb[0:2, :], seld[:])

        # ---------------- QKV projection ----------------
        for pair in range(NPAIR):
            for q in range(TQ):
                for colbase, dst in ((0, QT_sb), (512, KT_sb)):
                    ps = pss.tile([128, 512], F32, tag="ss")
                    for c in range(CCH):
                        nc.tensor.matmul(
                            ps[:],
                            wq_sb[:, c, colbase + pair * 128: colbase + (pair + 1) * 128],
                            xT_sb[:, c, q * 512:(q + 1) * 512],
                            start=(c == 0), stop=(c == CCH - 1),
                        )
                    nc.scalar.copy(dst[:, pair, q * 512:(q + 1) * 512], ps[:])
        for tt in range(NT):
            ps = pss.tile([128, 512], F32, tag="ss")
            for c in range(CCH):
                nc.tensor.matmul(
                    ps[:],
                    xT_sb[:, c, tt * 128:(tt + 1) * 128],
                    wq_sb[:, c, 1024:1536],
                    start=(c == 0), stop=(c == CCH - 1),
                )
            nc.scalar.copy(
                vr[:, tt, :, 0:64],
                ps[:].rearrange("p (h d) -> p h d", d=64),
            )

        # ---------------- attention ----------------
        for pair in range(NPAIR):
            hA, hB = 2 * pair, 2 * pair + 1
            for q in range(TQ):
                ya = psy.tile([VROW, 512], F32, tag="yA")
                yb = psy.tile([VROW, 512], F32, tag="yB")
                ntk = 4 * (q + 1)
                for tk in range(ntk):
                    ssm = pss.tile([128, 1024], F32, tag="ss")
                    nc.tensor.matmul(
                        ssm[:, 0:512],
                        KT_sb[0:64, pair, tk * 128:(tk + 1) * 128],
                        QT_sb[0:64, pair, q * 512:(q + 1) * 512],
                        start=True, stop=True,
                    )
                    nc.tensor.matmul(
                        ssm[:, 512:1024],
                        KT_sb[64:128, pair, tk * 128:(tk + 1) * 128],
                        QT_sb[64:128, pair, q * 512:(q + 1) * 512],
                        start=True, stop=True,
                    )
                    pm = ppool.tile([128, 1024], BF16, tag="P")
                    nc.scalar.activation(pm[:], ssm[:], AF.Exp)
                    if tk >= 4 * q:
                        off = (tk - 4 * q) * 512
                        nc.vector.tensor_mul(
                            pm[:, 0:512], pm[:, 0:512], dm_sb[:, off:off + 512])
                        nc.vector.tensor_mul(
                            pm[:, 512:1024], pm[:, 512:1024], dm_sb[:, off:off + 512])
                    first = (tk == 0)
                    last = (tk == ntk - 1)
                    nc.tensor.matmul(
                        ya[:], V_sb[:, tk, hA * VROW:(hA + 1) * VROW],
                        pm[:, 0:512],
                        start=first, stop=last,
                    )
                    nc.tensor.matmul(
                        yb[:], V_sb[:, tk, hB * VROW:(hB + 1) * VROW],
                        pm[:, 512:1024],
                        start=first, stop=last,
                    )
                # epilogue: strip denominators to DRAM staging, evac y
                dt = spool.tile([128, 1024], F32, tag="Dt")
                nc.vector.tensor_copy(dt[64:65, 0:512], ya[64:65, :])
                nc.vector.tensor_copy(dt[64:65, 512:1024], yb[64:65, :])
                nc.sync.dma_start(dstage[4 * pair + q, :], dt[64:65, :])
                nc.vector.tensor_copy(
                    Yu_sb[0:64, pair, q * 512:(q + 1) * 512], ya[0:64, :])
                bs = spool.tile([64, 512], BF16, tag="Bs")
                nc.vector.tensor_copy(bs[:], yb[0:64, :])
                nc.sync.dma_start(
                    Yu_sb[64:128, pair, q * 512:(q + 1) * 512], bs[:])
            # batched reciprocal of this pair's 4096 denominators ([128, 32])
            dp = spool.tile([128, 32], F32, tag="Dp")
            nc.sync.dma_start(
                dp[:],
                dstage[4 * pair:4 * pair + 4, :]
                .rearrange("q v -> (q v)").rearrange("(r c) -> r c", c=32))
            rp = spool.tile([128, 32], F32, tag="Rp")
            nc.vector.reciprocal(rp[:], dp[:])
            nc.sync.dma_start(
                rstage[4 * pair:4 * pair + 4, :]
                .rearrange("q v -> (q v)").rearrange("(r c) -> r c", c=32),
                rp[:])
        # normalize: k=2 PE broadcast of reciprocals + one in-place mul per pair
        # (kept out of the attention loop so the DMA/recip chain never blocks
        # the in-order PE stream between pairs)
        for pair in range(NPAIR):
            RB = rbpool.tile([128, T], F32, tag="RB")
            for q in range(TQ):
                rt = spool.tile([128, 512], F32R, tag="Rt")
                nc.sync.dma_start(
                    rt[0:2, :],
                    rstage[4 * pair + q, :]
                    .rearrange("(a c) -> a c", c=512).bitcast(F32R))
                bc = pss.tile([128, 512], F32, tag="ss")
                nc.tensor.matmul(
                    bc[:], sel_sb[0:2, :], rt[0:2, :], start=True, stop=True)
                nc.vector.tensor_copy(RB[:, q * 512:(q + 1) * 512], bc[:])
            nc.vector.tensor_mul(Yu_sb[:, pair, :], Yu_sb[:, pair, :], RB[:])

        # ---------------- output projection (partial over this core's heads) --
        for tt in range(NT):
            ot = opool.tile([128, C], F32, tag="Ot")
            for oc in range(2):
                po = pss.tile([128, 512], F32, tag="ss")
                for pair in range(NPAIR):
                    nc.tensor.matmul(
                        po[:],
                        Yu_sb[:, pair, tt * 128:(tt + 1) * 128],
                        wp_sb[:, pair, oc * 512:(oc + 1) * 512],
                        start=(pair == 0), stop=(pair == NPAIR - 1),
                    )
                nc.vector.tensor_copy(ot[:, oc * 512:(oc + 1) * 512], po[:])
            nc.sync.dma_start(out[tt * 128:(tt + 1) * 128, :], ot[:])

    return nc


def make_in_maps(x: np.ndarray, w_qkv: np.ndarray, w_proj: np.ndarray):
    bf = ml_dtypes.bfloat16
    scale = np.float32(DH ** -0.5)

    iq = np.arange(512)[None, :]
    ik = np.arange(128)[:, None]
    dmask = np.concatenate(
        [(iq >= j * 128 + ik) for j in range(4)], axis=1).astype(bf)

    in_maps = []
    for core in range(NCORES):
        b, g = core // 2, core % 2
        xTb = np.ascontiguousarray(x[b].T).astype(bf)           # [C, T]
        wq = (w_qkv[512 * g: 512 * g + 512] * scale).astype(np.float32)
        wk = w_qkv[1024 + 512 * g: 1024 + 512 * g + 512]
        wv = w_qkv[2048 + 512 * g: 2048 + 512 * g + 512]
        wqkvT = np.ascontiguousarray(
            np.concatenate([wq, wk, wv], axis=0).T).astype(bf)  # [C, 1536]
        wpT = np.ascontiguousarray(
            w_proj[:, 512 * g: 512 * g + 512].T).astype(bf)     # [512, C]
        sel = np.zeros((2, 128), dtype=np.float32)
        sel[0, 0:64] = 1.0
        sel[1, 64:128] = 1.0
        in_maps.append({"xT": xTb, "wqkvT": wqkvT, "wpT": wpT, "dmask": dmask,
                        "sel": sel})
    return in_maps


_NC = None


def kernel(x: np.ndarray, w_qkv: np.ndarray, w_proj: np.ndarray,
           _trace: bool = False, _return_raw: bool = False) -> np.ndarray:
    global _NC
    x = np.asarray(x, dtype=np.float32)
    w_qkv = np.asarray(w_qkv, dtype=np.float32)
    w_proj = np.asarray(w_proj, dtype=np.float32)
    if _NC is None:
        _NC = build_program()
    in_maps = make_in_maps(x, w_qkv, w_proj)
    res = run_bass_kernel_spmd(_NC, in_maps, list(range(NCORES)), trace=_trace)
    B = x.shape[0]
    outp = np.empty((B, T, C), dtype=np.float32)
    for b in range(B):
        outp[b] = res.results[2 * b]["out"] + res.results[2 * b + 1]["out"]
    if _return_raw:
        return outp, res
    return outp



# revision 21
# speedup vs baseline: 1.2213x; 1.2213x over previous
"""Causal self-attention (B=4, T=2048, C=1024, H=16) on 8 trn2 NeuronCores.

Sharding: core c -> (batch b = c//2, head-group g = c%2 of 8 heads).
Each core computes qkv projection, causal attention and the proj partial-sum
for its 8 heads on its batch; the host sums the two head-group partials per
batch (row-parallel linear unshard).

Per-core kernel layout (all matmuls bf16, f32 accumulation):
  Chunk-outer schedule: queries processed in 4 chunks of 512; per chunk all
  4 head-pairs run attention, then that chunk's outputs are transposed,
  projected and DMA'd out while the next chunk computes.  QK projection for
  chunk c+1 and V tiles are interleaved as PE filler inside chunk c so the
  tensor engine never waits on the (Activation-bound) exp stream.

  Scores S[t_k, t_q] per head pair via two k=64 matmuls in PE row-groups
  0-63/64-127, causally trimmed to query cols >= key tile.  Intra-tile
  triangle masking is done on the PE: a preload matmul (identity x step
  mask) writes -60 into the masked triangle of the PSUM score tile and the
  score matmul accumulates on top, so exp gives exact zeros (no DVE mask).
  exp on ScalarE over [128, 2, n] strided (both heads, trimmed).
  AV transposed: out[q_tile 128, 65] = P[k,q]^T . [V | 1]; the ones column
  lands the softmax denominator in PSUM col 64 per query partition, so
  normalization is a per-partition tensor_scalar at evacuation (DVE/Pool),
  after one strided reciprocal per pair.  Y^T tiles are transposed back to
  feature-major with PE transpose matmuls, then proj accumulates over pairs
  with k=128 chunks; bf16 output DMA'd per token tile (host sums in f32).
"""

from collections import deque
from contextlib import ExitStack

import ml_dtypes
import numpy as np
import orjson

import concourse.bass as bass
import concourse.mybir as mybir
import concourse.tile as tile
from concourse.bass_utils import run_bass_kernel_spmd

BF16 = mybir.dt.bfloat16
F32 = mybir.dt.float32
AF = mybir.ActivationFunctionType

T, C, H, DH = 2048, 1024, 16, 64
NCORES = 8
NPAIR = 4            # head pairs per core (8 heads)
CCH = C // 128       # contraction chunks for qkv
NT = T // 128        # token tiles
VROW = 65            # 64 v-cols + ones column

# --- walrus in this env accepts only ONE sync-wait per instruction: split
# extras onto preceding same-engine NoOps at the BIR-JSON level.
if not getattr(bass.Bass, "_ant_wait_split", False):
    _orig_to_json_bytes = bass.Bass.to_json_bytes

    def _to_json_split_waits(self):
        m = orjson.loads(_orig_to_json_bytes(self))
        for f in m.get("functions", []):
            for bb in f.get("blocks") or []:
                insts = bb.get("instructions") or []
                out, changed = [], False
                for inst in insts:
                    si = inst.get("sync_info")
                    waits = (si or {}).get("on_wait") or []
                    if len(waits) > 1:
                        for j, w in enumerate(waits[:-1]):
                            out.append({
                                "debug": inst.get("debug", 0),
                                "engine": inst["engine"],
                                "ins": [], "outs": [],
                                "name": f"{inst['name']}-sw{j}",
                                "opcode": "NoOp",
                                "sync_info": {"on_wait": [w], "on_update": []},
                            })
                        si["on_wait"] = waits[-1:]
                        changed = True
                    out.append(inst)
                if changed:
                    bb["instructions"] = out
        return orjson.dumps(m)

    bass.Bass.to_json_bytes = _to_json_split_waits
    bass.Bass._ant_wait_split = True


def build_program() -> bass.Bass:
    nc = bass.Bass()
    xT = nc.dram_tensor("xT", [C, T], BF16, kind="ExternalInput")
    wqkvT = nc.dram_tensor("wqkvT", [C, 1536], BF16, kind="ExternalInput")
    wpT = nc.dram_tensor("wpT", [512, C], BF16, kind="ExternalInput")
    smd = nc.dram_tensor("sm", [128, 512], BF16, kind="ExternalInput")
    idd = nc.dram_tensor("id", [128, 128], BF16, kind="ExternalInput")
    out = nc.dram_tensor("out", [T, C], BF16, kind="ExternalOutput")

    with ExitStack() as ctx:
        tc = ctx.enter_context(tile.TileContext(nc))
        const = ctx.enter_context(tc.tile_pool(name="const", bufs=1))
        pss = ctx.enter_context(tc.tile_pool(name="pss", bufs=2, space="PSUM"))
        pav = ctx.enter_context(tc.tile_pool(name="pav", bufs=1, space="PSUM"))
        pep = ctx.enter_context(tc.tile_pool(name="pep", bufs=2, space="PSUM"))
        pmp = ctx.enter_context(tc.tile_pool(name="pmp", bufs=4))
        rdp = ctx.enter_context(tc.tile_pool(name="rdp", bufs=2))
        yup = ctx.enter_context(tc.tile_pool(name="yup", bufs=2))
        otp = ctx.enter_context(tc.tile_pool(name="otp", bufs=2))

        xT_sb = const.tile([128, CCH, T], BF16, tag="xT")
        wq_sb = const.tile([128, CCH, 1536], BF16, tag="wq")
        wp_sb = const.tile([128, NPAIR, C], BF16, tag="wp")
        sm_sb = const.tile([128, 512], BF16, tag="sm")
        id_sb = const.tile([128, 128], BF16, tag="id")
        QT_sb = const.tile([128, NPAIR, T], BF16, tag="QT")
        KT_sb = const.tile([128, NPAIR, T], BF16, tag="KT")
        V_sb = const.tile([128, NT, 8 * VROW], BF16, tag="V")
        Yt_sb = const.tile([128, 4, 4, 512], BF16, tag="Yt")

        # ---------------- input DMA (consumption order) ----------------
        nc.sync.dma_start(sm_sb[:], smd[:])
        nc.sync.dma_start(id_sb[:], idd[:])
        for ci in range(CCH):
            nc.sync.dma_start(xT_sb[:, ci, :], xT[ci * 128:(ci + 1) * 128, :])
            nc.sync.dma_start(wq_sb[:, ci, 0:1024],
                              wqkvT[ci * 128:(ci + 1) * 128, 0:1024])
        for ci in range(CCH):
            nc.sync.dma_start(wq_sb[:, ci, 1024:1536],
                              wqkvT[ci * 128:(ci + 1) * 128, 1024:1536])
        for p in range(NPAIR):
            nc.sync.dma_start(wp_sb[:, p, :], wpT[p * 128:(p + 1) * 128, :])

        vr = V_sb[:].rearrange("p n (h e) -> p n h e", e=VROW)
        nc.gpsimd.memset(vr[:, :, :, 64:65], 1.0)

        # ---------------- filler jobs (PE work fed between attention ops) ---
        def qk_job(p, c, colbase, dst):
            def run():
                ps = pss.tile([128, 512], F32, tag="ss", name="psqk")
                for ci in range(CCH):
                    nc.tensor.matmul(
                        ps[:],
                        wq_sb[:, ci, colbase + p * 128: colbase + (p + 1) * 128],
                        xT_sb[:, ci, c * 512:(c + 1) * 512],
                        start=(ci == 0), stop=(ci == CCH - 1),
                    )
                nc.scalar.copy(dst[:, p, c * 512:(c + 1) * 512], ps[:])
            return run

        def v_job(tt):
            def run():
                ps = pss.tile([128, 512], F32, tag="ss", name="psv")
                for ci in range(CCH):
                    nc.tensor.matmul(
                        ps[:],
                        xT_sb[:, ci, tt * 128:(tt + 1) * 128],
                        wq_sb[:, ci, 1024:1536],
                        start=(ci == 0), stop=(ci == CCH - 1),
                    )
                nc.scalar.copy(
                    vr[:, tt, :, 0:64],
                    ps[:].rearrange("p (h d) -> p h d", d=64),
                )
            return run

        def tp_job(c, qt):
            def run():
                tr = pep.tile([128, 4, 128], BF16, tag="ep", name="tr")
                for p in range(NPAIR):
                    nc.tensor.transpose(
                        tr[:, p, :],
                        Yt_sb[:, c, qt, p * 128:(p + 1) * 128],
                        id_sb[:],
                    )
                yu = yup.tile([128, 4, 128], BF16, tag="yu", name="yu")
                nc.vector.tensor_copy(yu[:], tr[:])
                return yu
            return run

        yus = {}

        def proj_job(c, qt, oc, tpj=None):
            tt = 4 * c + qt

            def run():
                if tpj is not None:
                    yus[tt] = tpj()
                yu = yus[tt]
                po = pep.tile([128, 512], F32, tag="ep", name="po")
                for p in range(NPAIR):
                    nc.tensor.matmul(
                        po[:], yu[:, p, :], wp_sb[:, p, oc * 512:(oc + 1) * 512],
                        start=(p == 0), stop=(p == NPAIR - 1),
                    )
                ot = otp.tile([128, 512], BF16, tag="ot", name="ot")
                nc.vector.tensor_copy(ot[:], po[:])
                nc.sync.dma_start(
                    out[tt * 128:(tt + 1) * 128, oc * 512:(oc + 1) * 512], ot[:])
            return run

        def endphase_jobs(c):
            jobs = []
            for qt in range(4):
                jobs.append(proj_job(c, qt, 0, tpj=tp_job(c, qt)))
                jobs.append(proj_job(c, qt, 1))
            return jobs

        # ---------------- preamble: QK chunk 0, V tiles 0..3 ----------------
        for p in range(NPAIR):
            qk_job(p, 0, 0, QT_sb)()
            qk_job(p, 0, 512, KT_sb)()
        for tt in range(4):
            v_job(tt)()

        # ---------------- attention: unit schedule ----------------
        # Units (chunk, pair) in an order that interleaves the Act-heavy c3
        # units with c2 ones so the exp stream never outruns the PE.  Filler
        # PE jobs (QK/V for later chunks, transpose+proj+out for finished
        # chunks) are assigned per-unit and fed evenly across its kt sweep.
        def qkv_fillers(c):
            jobs = [qk_job(p, c, cb, dst)
                    for p in range(NPAIR)
                    for cb, dst in ((0, QT_sb), (512, KT_sb))]
            jobs += [v_job(tt) for tt in range(4 * c, 4 * c + 4)]
            return jobs

        c0f = qkv_fillers(1)
        unit_fill = {
            (0, 0): c0f[:3], (0, 1): c0f[3:6],
            (0, 2): c0f[6:9], (0, 3): c0f[9:],
        }
        c1f = qkv_fillers(2) + qkv_fillers(3) + endphase_jobs(0)
        for p in range(4):
            unit_fill[(1, p)] = c1f[p * 8:(p + 1) * 8]
        duos = [(2, 0), (3, 0), (2, 1), (3, 1), (2, 2), (3, 2), (2, 3), (3, 3)]
        e1 = endphase_jobs(1)
        for i, u in enumerate(duos[:7]):
            unit_fill[u] = e1[i:i + 2] if i == 6 else e1[i:i + 1]
        unit_fill[duos[6]] = e1[6:8]
        unit_fill[(3, 3)] = endphase_jobs(2)

        schedule = [(0, 0), (0, 1), (0, 2), (0, 3),
                    (1, 0), (1, 1), (1, 2), (1, 3)] + duos

        for c, pair in schedule:
            fill = deque(unit_fill.get((c, pair), []))
            nkt = 4 * c + 4
            total_iters = nkt
            njobs = len(fill)
            fed = 0
            it = 0
            if True:
                av = pav.tile([128, 4, 256], F32, tag="av", name="av")
                pml = {}

                def emit_scores(kt):
                    ssm = pss.tile([128, 1024], F32, tag="ss", name="ssm")
                    q0 = max(0, 128 * (kt - 4 * c))
                    diag = kt >= 4 * c
                    for h in range(2):
                        base = 512 * h
                        lk = KT_sb[64 * h:64 * h + 64, pair,
                                   kt * 128:(kt + 1) * 128]
                        if diag:
                            # one accumulation group per head-bank: preload
                            # starts (clears has_written bank-wide), the diag
                            # block accumulates on it, the clean tail
                            # overwrites (bits cleared, never written).
                            has_clean = q0 + 128 < 512
                            nc.tensor.matmul(
                                ssm[:, base + q0:base + q0 + 128],
                                id_sb[:], sm_sb[:, 0:128],
                                start=True, stop=False,
                            )
                            nc.tensor.matmul(
                                ssm[:, base + q0:base + q0 + 128],
                                lk,
                                QT_sb[64 * h:64 * h + 64, pair,
                                      c * 512 + q0:c * 512 + q0 + 128],
                                start=False, stop=not has_clean,
                            )
                            if has_clean:
                                nc.tensor.matmul(
                                    ssm[:, base + q0 + 128:base + 512],
                                    lk,
                                    QT_sb[64 * h:64 * h + 64, pair,
                                          c * 512 + q0 + 128:(c + 1) * 512],
                                    start=False, stop=True,
                                )
                        else:
                            nc.tensor.matmul(
                                ssm[:, base:base + 512],
                                lk,
                                QT_sb[64 * h:64 * h + 64, pair,
                                      c * 512:(c + 1) * 512],
                                start=True, stop=True,
                            )
                    pmt = pmp.tile([128, 1024], BF16, tag="P", name="P")
                    nc.scalar.activation(
                        pmt.rearrange("p (h n) -> p h n", h=2)[:, :, q0:512],
                        ssm.rearrange("p (h n) -> p h n", h=2)[:, :, q0:512],
                        AF.Exp,
                    )
                    pml[kt] = pmt

                def emit_av(kt):
                    # av packs qt pairs {0,1} and {2,3} per PSUM bank: ONE
                    # accumulation group per bank (start clears has_written
                    # bank-wide; each region's first touch overwrites, later
                    # kts accumulate; stop on the bank's last matmul).
                    pmt = pml.pop(kt)
                    for qt in range(max(0, kt - 4 * c), 4):
                        for h in range(2):
                            nc.tensor.matmul(
                                av[:, qt, VROW * h:VROW * h + VROW],
                                pmt[:, 512 * h + 128 * qt:512 * h + 128 * qt + 128],
                                V_sb[:, kt, (2 * pair + h) * VROW:
                                     (2 * pair + h + 1) * VROW],
                                start=(kt == 0 and h == 0 and qt % 2 == 0),
                                stop=(h == 1 and qt % 2 == 1
                                      and kt == 4 * c + qt),
                            )

                for kt in range(nkt):
                    emit_scores(kt)
                    # spread filler jobs evenly across the chunk's kt sweep
                    it += 1
                    while fill and fed < (it * njobs) // total_iters:
                        fill.popleft()()
                        fed += 1
                    if kt > 1:
                        emit_av(kt - 2)
                if nkt > 1:
                    emit_av(nkt - 2)
                emit_av(nkt - 1)

                rd = rdp.tile([128, 4, 2], F32, tag="rd", name="rd")
                nc.vector.reciprocal(rd[:], av[:, :, 64:130:65])
                for qt in range(4):
                    for h in range(2):
                        nc.vector.tensor_scalar_mul(
                            Yt_sb[:, c, qt,
                                  (2 * pair + h) * 64:(2 * pair + h + 1) * 64],
                            av[:, qt, VROW * h:VROW * h + 64],
                            rd[:, qt, h:h + 1],
                        )
            while fill:
                fill.popleft()()
        for job in endphase_jobs(3):
            job()

    return nc


def make_in_maps(x: np.ndarray, w_qkv: np.ndarray, w_proj: np.ndarray):
    bf = ml_dtypes.bfloat16
    scale = np.float32(DH ** -0.5)

    ik = np.arange(128)[:, None]
    ij = np.arange(512)[None, :]
    sm = np.where(ik > ij, np.float32(-60.0), np.float32(0.0)).astype(bf)
    iden = np.eye(128, dtype=bf)

    in_maps = []
    for core in range(NCORES):
        b, g = core // 2, core % 2
        xTb = np.ascontiguousarray(x[b].T).astype(bf)           # [C, T]
        wq = (w_qkv[512 * g: 512 * g + 512] * scale).astype(np.float32)
        wk = w_qkv[1024 + 512 * g: 1024 + 512 * g + 512]
        wv = w_qkv[2048 + 512 * g: 2048 + 512 * g + 512]
        wqkvT = np.ascontiguousarray(
            np.concatenate([wq, wk, wv], axis=0).T).astype(bf)  # [C, 1536]
        wpT = np.ascontiguousarray(
            w_proj[:, 512 * g: 512 * g + 512].T).astype(bf)     # [512, C]
        in_maps.append({"xT": xTb, "wqkvT": wqkvT, "wpT": wpT,
                        "sm": sm, "id": iden})
    return in_maps


_NC = None


def kernel(x: np.ndarray, w_qkv: np.ndarray, w_proj: np.ndarray,
           _trace: bool = False, _return_raw: bool = False) -> np.ndarray:
    global _NC
    x = np.asarray(x, dtype=np.float32)
    w_qkv = np.asarray(w_qkv, dtype=np.float32)
    w_proj = np.asarray(w_proj, dtype=np.float32)
    if _NC is None:
        _NC = build_program()
    in_maps = make_in_maps(x, w_qkv, w_proj)
    res = run_bass_kernel_spmd(_NC, in_maps, list(range(NCORES)), trace=_trace)
    B = x.shape[0]
    outp = np.empty((B, T, C), dtype=np.float32)
    for b in range(B):
        outp[b] = (res.results[2 * b]["out"].astype(np.float32)
                   + res.results[2 * b + 1]["out"].astype(np.float32))
    if _return_raw:
        return outp, res
    return outp


# revision 60
# speedup vs baseline: 1.3959x; 1.1430x over previous
"""Causal self-attention (B=4, T=2048, C=1024, H=16) on 8 trn2 NeuronCores.

Sharding: core c -> (batch b = c//2, head-group g = c%2 of 8 heads).
Each core computes qkv projection, causal attention and the proj partial-sum
for its 8 heads on its batch; the host sums the two head-group partials per
batch (row-parallel linear unshard).

Per-core kernel layout (all matmuls bf16, f32 accumulation):
  Chunk-outer schedule: queries processed in 4 chunks of 512; per chunk all
  4 head-pairs run attention, then that chunk's outputs are transposed,
  projected and DMA'd out while the next chunk computes.  QK projection for
  chunk c+1 and V tiles are interleaved as PE filler inside chunk c so the
  tensor engine never waits on the (Activation-bound) exp stream.

  Scores S[t_k, t_q] per head pair via two k=64 matmuls in PE row-groups
  0-63/64-127, causally trimmed to query cols >= key tile.  Intra-tile
  triangle masking is done on the PE: a preload matmul (identity x step
  mask) writes -60 into the masked triangle of the PSUM score tile and the
  score matmul accumulates on top, so exp gives exact zeros (no DVE mask).
  exp on ScalarE over [128, 2, n] strided (both heads, trimmed).
  AV transposed: out[q_tile 128, 65] = P[k,q]^T . [V | 1]; the ones column
  lands the softmax denominator in PSUM col 64 per query partition, so
  normalization is a per-partition tensor_scalar at evacuation (DVE/Pool),
  after one strided reciprocal per pair.  Y^T tiles are transposed back to
  feature-major with PE transpose matmuls, then proj accumulates over pairs
  with k=128 chunks; bf16 output DMA'd per token tile (host sums in f32).
"""

from collections import deque
from contextlib import ExitStack

import ml_dtypes
import numpy as np
import orjson

import concourse.bass as bass
import concourse.mybir as mybir
import concourse.tile as tile
from concourse.bass_utils import run_bass_kernel_spmd

BF16 = mybir.dt.bfloat16
F32 = mybir.dt.float32
AF = mybir.ActivationFunctionType

T, C, H, DH = 2048, 1024, 16, 64
NCORES = 8
NPAIR = 4            # head pairs per core (8 heads)
CCH = C // 128       # contraction chunks for qkv
NT = T // 128        # token tiles
VROW = 65            # 64 v-cols + ones column

# --- walrus in this env accepts only ONE sync-wait per instruction: split
# extras onto preceding same-engine NoOps at the BIR-JSON level.
if not getattr(bass.Bass, "_ant_wait_split", False):
    _orig_to_json_bytes = bass.Bass.to_json_bytes

    def _to_json_split_waits(self):
        m = orjson.loads(_orig_to_json_bytes(self))
        for f in m.get("functions", []):
            for bb in f.get("blocks") or []:
                insts = bb.get("instructions") or []
                out, changed = [], False
                for inst in insts:
                    si = inst.get("sync_info")
                    waits = (si or {}).get("on_wait") or []
                    if len(waits) > 1:
                        for j, w in enumerate(waits[:-1]):
                            out.append({
                                "debug": inst.get("debug", 0),
                                "engine": inst["engine"],
                                "ins": [], "outs": [],
                                "name": f"{inst['name']}-sw{j}",
                                "opcode": "NoOp",
                                "sync_info": {"on_wait": [w], "on_update": []},
                            })
                        si["on_wait"] = waits[-1:]
                        changed = True
                    out.append(inst)
                if changed:
                    bb["instructions"] = out
        return orjson.dumps(m)

    bass.Bass.to_json_bytes = _to_json_split_waits
    bass.Bass._ant_wait_split = True


def build_program() -> bass.Bass:
    nc = bass.Bass()
    xT = nc.dram_tensor("xT", [C, T], BF16, kind="ExternalInput")
    wqkvT = nc.dram_tensor("wqkvT", [C, 1536], BF16, kind="ExternalInput")
    wpT = nc.dram_tensor("wpT", [512, C], BF16, kind="ExternalInput")
    smd = nc.dram_tensor("sm", [128, 128], BF16, kind="ExternalInput")
    idd = nc.dram_tensor("id", [128, 128], BF16, kind="ExternalInput")
    out = nc.dram_tensor("out", [T, C], BF16, kind="ExternalOutput")

    with ExitStack() as ctx:
        tc = ctx.enter_context(tile.TileContext(nc))
        const = ctx.enter_context(tc.tile_pool(name="const", bufs=1))
        pss = ctx.enter_context(tc.tile_pool(name="pss", bufs=2, space="PSUM"))
        pav = ctx.enter_context(tc.tile_pool(name="pav", bufs=1, space="PSUM"))
        # "qk" carries the sliced QK-projection fillers of the S-stream;
        # "ep" carries the (FIFO) AV-side fillers: V tiles, transposes (as a
        # bf16 view of the proj psum) and proj outputs.  The two streams
        # interleave, so they must not share a rotating slot.
        pqk = ctx.enter_context(tc.tile_pool(name="pqk", bufs=1, space="PSUM"))
        pep = ctx.enter_context(tc.tile_pool(name="pep", bufs=1, space="PSUM"))
        pmp = ctx.enter_context(tc.tile_pool(name="pmp", bufs=37))
        rdp = ctx.enter_context(tc.tile_pool(name="rdp", bufs=2))
        yup = ctx.enter_context(tc.tile_pool(name="yup", bufs=2))
        otp = ctx.enter_context(tc.tile_pool(name="otp", bufs=2))

        xT_sb = const.tile([128, CCH, T], BF16, tag="xT")
        wq_sb = const.tile([128, CCH, 1536], BF16, tag="wq")
        wp_sb = const.tile([128, NPAIR, C], BF16, tag="wp")
        sm_sb = const.tile([128, 128], BF16, tag="sm")
        id_sb = const.tile([128, 128], BF16, tag="id")
        QT_sb = const.tile([128, NPAIR, T], BF16, tag="QT")
        KT_sb = const.tile([128, NPAIR, T], BF16, tag="KT")
        V_sb = const.tile([128, NT, 8 * VROW], BF16, tag="V")
        Yt_sb = const.tile([128, 4, 4, 512], BF16, tag="Yt")

        # ---------------- input DMA (consumption order) ----------------
        nc.sync.dma_start(sm_sb[:], smd[:])
        nc.sync.dma_start(id_sb[:], idd[:])
        for ci in range(CCH):
            nc.sync.dma_start(xT_sb[:, ci, :], xT[ci * 128:(ci + 1) * 128, :])
            nc.sync.dma_start(wq_sb[:, ci, 0:1024],
                              wqkvT[ci * 128:(ci + 1) * 128, 0:1024])
        for ci in range(CCH):
            nc.sync.dma_start(wq_sb[:, ci, 1024:1536],
                              wqkvT[ci * 128:(ci + 1) * 128, 1024:1536])
        for p in range(NPAIR):
            nc.sync.dma_start(wp_sb[:, p, :], wpT[p * 128:(p + 1) * 128, :])

        vr = V_sb[:].rearrange("p n (h e) -> p n h e", e=VROW)
        nc.gpsimd.memset(vr[:, :, :, 64:65], 1.0)
        # warm the Exp activation table during the DMA ramp
        warm = rdp.tile([1, 2], F32, tag="warm", name="warm")
        nc.scalar.activation(warm[:], sm_sb[0:1, 0:2], AF.Exp)

        # ---------------- filler jobs (PE work fed between attention ops) ---
        # Fillers are generators yielding every ~2 matmuls so a long
        # projection never blocks the in-order PE stream between the
        # Act-paced score steps.
        def qk_job(p, c, colbase, dst, sliced=True):
            def run():
                ps = pqk.tile([128, 512], F32, tag="qk", name="psqk")
                for ci in range(CCH):
                    nc.tensor.matmul(
                        ps[:],
                        wq_sb[:, ci, colbase + p * 128: colbase + (p + 1) * 128],
                        xT_sb[:, ci, c * 512:(c + 1) * 512],
                        start=(ci == 0), stop=(ci == CCH - 1),
                    )
                    if sliced and ci % 2 == 1 and ci < CCH - 1:
                        yield
                if c <= 1:
                    nc.scalar.copy(dst[:, p, c * 512:(c + 1) * 512], ps[:])
                else:
                    nc.vector.tensor_copy(
                        dst[:, p, c * 512:(c + 1) * 512], ps[:])
            return run

        def v_job(tt):
            def run():
                ps = pep.tile([128, 512], F32, tag="ep", name="psv")
                for ci in range(CCH):
                    nc.tensor.matmul(
                        ps[:],
                        xT_sb[:, ci, tt * 128:(tt + 1) * 128],
                        wq_sb[:, ci, 1024:1536],
                        start=(ci == 0), stop=(ci == CCH - 1),
                    )
                    if ci == 3:
                        yield
                nc.vector.tensor_copy(
                    vr[:, tt, :, 0:64],
                    ps[:].rearrange("p (h d) -> p h d", d=64),
                )
            return run

        def drain(gen):
            for _ in gen:
                pass

        yus = {}

        def proj_job(c, qt, oc, with_tp=False, act_evac=False, slot2=False):
            tt = 4 * c + qt

            def run():
                # the tail endphases borrow the (then-idle) qk psum bank for
                # the odd halves so consecutive proj outputs double-buffer
                if slot2:
                    po = pqk.tile([128, 512], F32, tag="qk", name="po2")
                else:
                    po = pep.tile([128, 512], F32, tag="ep", name="po")
                if with_tp:
                    # transpose Y^T back to feature-major THROUGH the proj
                    # psum tile (bf16 view); evacuated to SBUF before the
                    # proj matmuls overwrite the bank.
                    trv = po[:].bitcast(BF16)     # [128, 1024] bf16
                    for p in range(NPAIR):
                        nc.tensor.transpose(
                            trv[:, p * 128:(p + 1) * 128],
                            Yt_sb[:, c, qt, p * 128:(p + 1) * 128],
                            id_sb[:],
                        )
                    yu = yup.tile([128, 4, 128], BF16, tag="yu", name="yu")
                    nc.vector.tensor_copy(
                        yu[:], trv[:, 0:512].rearrange("p (q x) -> p q x", x=128))
                    yus[tt] = yu
                    yield
                yu = yus[tt]
                for p in range(NPAIR):
                    nc.tensor.matmul(
                        po[:], yu[:, p, :], wp_sb[:, p, oc * 512:(oc + 1) * 512],
                        start=(p == 0), stop=(p == NPAIR - 1),
                    )
                    if p == 1:
                        yield
                ot = otp.tile([128, 512], BF16, tag="ot", name="ot")
                if act_evac:
                    nc.scalar.copy(ot[:], po[:])
                else:
                    nc.vector.tensor_copy(ot[:], po[:])
                nc.sync.dma_start(
                    out[tt * 128:(tt + 1) * 128, oc * 512:(oc + 1) * 512], ot[:])
            return run

        def endphase_jobs(c, act_evac=False):
            jobs = []
            for qt in range(4):
                jobs.append(proj_job(c, qt, 0, with_tp=True,
                                     act_evac=act_evac))
                jobs.append(proj_job(c, qt, 1, act_evac=act_evac,
                                     slot2=act_evac))
            return jobs

        # ---------------- preamble: QK chunk 0, V tiles 0..3 ----------------
        for p in range(NPAIR):
            drain(qk_job(p, 0, 0, QT_sb)())
            drain(qk_job(p, 0, 512, KT_sb)())
        for tt in range(4):
            drain(v_job(tt)())

        # ---------------- attention: unit schedule ----------------
        # Units (chunk, pair) in an order that interleaves the Act-heavy c3
        # units with c2 ones so the exp stream never outruns the PE.  Filler
        # PE jobs (QK/V for later chunks, transpose+proj+out for finished
        # chunks) are assigned per-unit and fed evenly across its kt sweep.
        def qk_fillers(c):
            return [qk_job(p, c, cb, dst)
                    for p in range(NPAIR)
                    for cb, dst in ((0, QT_sb), (512, KT_sb))]

        duos = [(2, 0), (3, 0), (2, 1), (3, 1), (2, 2), (3, 2), (2, 3), (3, 3)]
        schedule = [(0, 0), (0, 1), (0, 2), (0, 3),
                    (1, 0), (1, 1), (1, 2), (1, 3)] + duos

        # QK projections for chunk c+1 are prerequisites of chunk-c+1
        # S-steps: key them to the S-stream of chunk c.
        s_fill_map = {}
        for c in range(3):
            jobs = qk_fillers(c + 1)
            for p in range(4):
                s_fill_map[(c, p)] = jobs[p * 2:(p + 1) * 2]
        # V tiles (needed by AV) and finished-chunk endphases are keyed to
        # the AV-stream.
        unit_fill = {(0, p): [v_job(4 + p)] for p in range(4)}
        for p in range(4):
            unit_fill[(1, p)] = [v_job(8 + p), v_job(12 + p)]
        e01 = endphase_jobs(0) + endphase_jobs(1)
        for i, u in enumerate(duos[:7]):
            lo = (i * 16) // 7
            hi = ((i + 1) * 16) // 7
            unit_fill[u] = e01[lo:hi]
        unit_fill[(3, 3)] = endphase_jobs(2, act_evac=True)
        # endphase(3) may only be emitted after (3,3)'s epilogues exist:
        # qt 0/1 after epi(bank 0) (kt 13), qt 2/3 after epi(bank 1) (kt 15).
        e3 = endphase_jobs(3, act_evac=True)
        late_fill = {(3, 3, 13): e3[:4], (3, 3, 15): e3[4:]}

        class Unit:
            def __init__(self, c, pair):
                self.c, self.pair, self.nkt = c, pair, 4 * c + 4
                self.av = None
                self.pml = {}

            def s_step(self, kt):
                c, pair = self.c, self.pair
                ssm = pss.tile([128, 1024], F32, tag="ss", name="ssm")
                q0 = max(0, 128 * (kt - 4 * c))
                diag = kt >= 4 * c
                for h in range(2):
                    base = 512 * h
                    lk = KT_sb[64 * h:64 * h + 64, pair,
                               kt * 128:(kt + 1) * 128]
                    if diag:
                        # one accumulation group per head-bank: preload
                        # starts (clears has_written bank-wide), the diag
                        # block accumulates on it, the clean tail overwrites
                        # (bits cleared by the start, never since written).
                        has_clean = q0 + 128 < 512
                        nc.tensor.matmul(
                            ssm[:, base + q0:base + q0 + 128],
                            id_sb[:], sm_sb[:, 0:128],
                            start=True, stop=False,
                        )
                        nc.tensor.matmul(
                            ssm[:, base + q0:base + q0 + 128],
                            lk,
                            QT_sb[64 * h:64 * h + 64, pair,
                                  c * 512 + q0:c * 512 + q0 + 128],
                            start=False, stop=not has_clean,
                        )
                        if has_clean:
                            nc.tensor.matmul(
                                ssm[:, base + q0 + 128:base + 512],
                                lk,
                                QT_sb[64 * h:64 * h + 64, pair,
                                      c * 512 + q0 + 128:(c + 1) * 512],
                                start=False, stop=True,
                            )
                    else:
                        nc.tensor.matmul(
                            ssm[:, base:base + 512],
                            lk,
                            QT_sb[64 * h:64 * h + 64, pair,
                                  c * 512:(c + 1) * 512],
                            start=True, stop=True,
                        )
                pmt = pmp.tile([128, 1024], BF16, tag="P", name="P")
                nc.scalar.activation(
                    pmt.rearrange("p (h n) -> p h n", h=2)[:, :, q0:512],
                    ssm.rearrange("p (h n) -> p h n", h=2)[:, :, q0:512],
                    AF.Exp,
                )
                self.pml[kt] = pmt

            def epi(self, bank):
                # per-bank epilogue right after its group stops, so the av
                # slot frees as early as possible.
                c, pair, av = self.c, self.pair, self.av
                rd = rdp.tile([128, 2, 2], F32, tag="rd", name="rd")
                nc.vector.reciprocal(
                    rd[:], av[:, 2 * bank:2 * bank + 2, 64:130:65])
                for qi in range(2):
                    qt = 2 * bank + qi
                    for h in range(2):
                        nc.vector.tensor_scalar_mul(
                            Yt_sb[:, c, qt,
                                  (2 * pair + h) * 64:(2 * pair + h + 1) * 64],
                            av[:, qt, VROW * h:VROW * h + 64],
                            rd[:, qi, h:h + 1],
                        )

            def av_step(self, kt):
                # av packs qt pairs {0,1} and {2,3} per PSUM bank: ONE
                # accumulation group per bank (start clears has_written
                # bank-wide; each region's first touch overwrites, later
                # kts accumulate; stop on the bank's last matmul).
                c, pair = self.c, self.pair
                if kt == 0:
                    self.av = pav.tile([128, 4, 256], F32, tag="av", name="av")
                pmt = self.pml.pop(kt)
                for qt in range(max(0, kt - 4 * c), 4):
                    for h in range(2):
                        nc.tensor.matmul(
                            self.av[:, qt, VROW * h:VROW * h + VROW],
                            pmt[:, 512 * h + 128 * qt:512 * h + 128 * qt + 128],
                            V_sb[:, kt, (2 * pair + h) * VROW:
                                 (2 * pair + h + 1) * VROW],
                            start=(kt == 0 and h == 0 and qt % 2 == 0),
                            stop=(h == 1 and qt % 2 == 1 and kt == 4 * c + qt),
                        )
                if kt == 4 * c + 1:
                    self.epi(0)
                elif kt == 4 * c + 3:
                    self.epi(1)

        # Two decoupled streams over the same (unit, kt) step list: the
        # S-stream (scores+exp) runs up to LAG kt-steps ahead of the
        # AV-stream, buffering P tiles in SBUF, so the Act engine is fed
        # through PE-heavy phases and AV never waits on exp.  Filler jobs
        # are keyed to AV-stream progress.
        units = {u: Unit(*u) for u in schedule}
        steps = [(units[u], kt) for u in schedule for kt in range(4 * u[0] + 4)]
        LAG0 = 34
        si = ai = 0
        s_fill = deque()
        a_fill = deque()
        av_flush_chunks = {1, 2}

        def feed(dq, n=1):
            while dq and n > 0:
                try:
                    next(dq[0])
                    n -= 1
                except StopIteration:
                    dq.popleft()

        def flush(dq):
            while dq:
                try:
                    next(dq[0])
                except StopIteration:
                    dq.popleft()

        s_unit_prev = av_unit_prev = None
        while ai < len(steps):
            # taper the S-lead near the stream end so the post-S av-drain
            # (PE with no Act overlap) stays short
            LAG = min(LAG0, max(8, len(steps) - 8 - ai))
            if si < len(steps) and si - ai < LAG:
                u, kt = steps[si]
                if u is not s_unit_prev:
                    if s_unit_prev is not None and u.c != s_unit_prev.c:
                        flush(s_fill)  # QK(c) must precede chunk-c S-steps
                    s_fill.extend(
                        g() for g in s_fill_map.get((u.c, u.pair), []))
                    s_unit_prev = u
                u.s_step(kt)
                si += 1
                feed(s_fill, 2 if u.c == 0 else 1)
            else:
                u, kt = steps[ai]
                if u is not av_unit_prev:
                    if u.c in av_flush_chunks:
                        av_flush_chunks.discard(u.c)
                        flush(a_fill)  # V tiles must precede their av chunk
                    a_fill.extend(
                        g() for g in unit_fill.get((u.c, u.pair), []))
                    av_unit_prev = u
                u.av_step(kt)
                a_fill.extend(
                    g() for g in late_fill.get((u.c, u.pair, kt), []))
                ai += 1
                feed(a_fill, 1 if si < len(steps) else 2)
        flush(s_fill)
        flush(a_fill)

    return nc


def make_in_maps(x: np.ndarray, w_qkv: np.ndarray, w_proj: np.ndarray):
    bf = ml_dtypes.bfloat16
    scale = np.float32(DH ** -0.5)

    ik = np.arange(128)[:, None]
    ij = np.arange(128)[None, :]
    sm = np.where(ik > ij, np.float32(-60.0), np.float32(0.0)).astype(bf)
    iden = np.eye(128, dtype=bf)

    in_maps = []
    for core in range(NCORES):
        b, g = core // 2, core % 2
        xTb = np.ascontiguousarray(x[b].T).astype(bf)           # [C, T]
        wq = (w_qkv[512 * g: 512 * g + 512] * scale).astype(np.float32)
        wk = w_qkv[1024 + 512 * g: 1024 + 512 * g + 512]
        wv = w_qkv[2048 + 512 * g: 2048 + 512 * g + 512]
        wqkvT = np.ascontiguousarray(
            np.concatenate([wq, wk, wv], axis=0).T).astype(bf)  # [C, 1536]
        wpT = np.ascontiguousarray(
            w_proj[:, 512 * g: 512 * g + 512].T).astype(bf)     # [512, C]
        in_maps.append({"xT": xTb, "wqkvT": wqkvT, "wpT": wpT,
                        "sm": sm, "id": iden})
    return in_maps


_NC = None


def kernel(x: np.ndarray, w_qkv: np.ndarray, w_proj: np.ndarray,
           _trace: bool = False, _return_raw: bool = False) -> np.ndarray:
    global _NC
    x = np.asarray(x, dtype=np.float32)
    w_qkv = np.asarray(w_qkv, dtype=np.float32)
    w_proj = np.asarray(w_proj, dtype=np.float32)
    if _NC is None:
        _NC = build_program()
    in_maps = make_in_maps(x, w_qkv, w_proj)
    res = run_bass_kernel_spmd(_NC, in_maps, list(range(NCORES)), trace=_trace)
    B = x.shape[0]
    outp = np.empty((B, T, C), dtype=np.float32)
    for b in range(B):
        outp[b] = (res.results[2 * b]["out"].astype(np.float32)
                   + res.results[2 * b + 1]["out"].astype(np.float32))
    if _return_raw:
        return outp, res
    return outp
